# revision 1
# baseline (speedup 1.0000x reference)
"""FFTConv2d kernel for trn2, 8 NeuronCores.

Math: reference einsum 'bchw,oihw->bohw' factorizes:
  Y[b,o] = conv_full(sum_c x[b,c], sum_i w[o,i])[1:-1,1:-1] + bias[o]
i.e. a single-channel 3x3 "same" convolution (flipped kernel) per (b,o).

Per core (2 batches):
  1. DMA x slice in as bf16 hi/lo pair (exact fp32 split), packed so each
     slice is one contiguous DMA; partitions=(b,c).
  2. Channel-sum via PE matmul with ones-indicator lhsT -> PSUM [6, n]
     (3 replicated copies per batch), accumulating hi+lo passes.
  3. Copy PSUM -> padded staging SBUF [6, 34*130] (row stride 130, zero
     borders), rounding to fp32r.
  4. Build P3 [8, 34*130]: partition (b,g) = staging col-shifted by (2-g);
     one contiguous SBUF->SBUF DMA each. Partitions (b,3) hold ones (bias).
  5. Conv: per 3-row output chunk, 3 accumulating fp32r matmuls (one per
     kernel row j) with rhs offset (2-j)*130 into P3 -> PSUM [128, 3, 130];
     all (b,o) images at once; bias rides the j=0 matmul's ones row.
  6. Copy PSUM -> Y SBUF (dropping the 2 pad columns per 130-row),
     DMA Y -> HBM.
Processed in NS row-slices for DMA/compute overlap.
"""

import os
import sys
from functools import lru_cache

import numpy as np

for _p in ("/opt/trn_rl_repo", "/root/.axon_site/_ro/trn_rl_repo"):
    if os.path.isdir(_p) and _p not in sys.path:
        sys.path.insert(0, _p)

import ml_dtypes

B, CIN, COUT, H, W = 16, 64, 64, 128, 128
N_CORES = 8
BPC = B // N_CORES  # batches per core = 2
NS = 4  # row slices per core
SH = H // NS  # rows per slice = 32
WROW = W + 2  # padded row stride = 130
PWIN = SH * WROW  # conv output window per slice = 4160
P3LEN = PWIN + 2 * WROW  # P3 length = 4420
SPLEN = P3LEN + 2  # staging length = 4422
NPART = BPC * CIN  # 128 input partitions (b, c)
NOUT = BPC * COUT  # 128 output partitions (b, o)
RMAX = SH + 2


def _slice_rows(s):
    h0 = max(0, SH * s - 1)
    he = min(H, SH * s + SH + 1)
    return h0, he


# packed input layout: per slice [hi rows | lo rows], contiguous
_SLICE_OFF = []
_off = 0
for _s in range(NS):
    _h0, _he = _slice_rows(_s)
    _SLICE_OFF.append(_off)
    _off += 2 * (_he - _h0) * W
XPACK_LEN = _off


@lru_cache(maxsize=1)
def _build():
    import concourse.bacc as bacc
    import concourse.mybir as mybir
    import concourse.tile as tile
    from concourse.ap import AP

    f32 = mybir.dt.float32
    f32r = mybir.dt.float32r
    bf16 = mybir.dt.bfloat16

    nc = bacc.Bacc("TRN2", target_bir_lowering=False, debug=False, num_devices=N_CORES)

    xp = nc.dram_tensor("xpack", [NPART, XPACK_LEN], bf16, kind="ExternalInput")
    ones_cs = nc.dram_tensor("ones_cs", [NPART, BPC * 3], bf16, kind="ExternalInput")
    wb = nc.dram_tensor("wb", [BPC * 9 + 1, NOUT], f32r, kind="ExternalInput")
    ones_p = nc.dram_tensor("ones_p", [1, PWIN], f32r, kind="ExternalInput")
    y = nc.dram_tensor("y", [NOUT, H * W], f32, kind="ExternalOutput")

    with tile.TileContext(nc) as tc:
        with (
            tc.tile_pool(name="xin", bufs=4) as xin_pool,
            tc.tile_pool(name="sp", bufs=1) as sp_pool,
            tc.tile_pool(name="pbuf", bufs=1) as p_pool,
            tc.tile_pool(name="yout", bufs=2) as y_pool,
            tc.tile_pool(name="consts", bufs=1) as c_pool,
            tc.tile_pool(name="cs_ps", bufs=4, space="PSUM") as cs_psum,
            tc.tile_pool(name="cv_ps", bufs=4, space="PSUM") as cv_psum,
        ):
            ones_t = c_pool.tile([NPART, BPC * 3], bf16, tag="ones_cs")
            nc.scalar.dma_start(out=ones_t[:, :], in_=ones_cs.ap()[:, :])
            wb_t = c_pool.tile([BPC * 9 + 1, NOUT], f32r, tag="wb")
            nc.scalar.dma_start(out=wb_t[:, :], in_=wb.ap()[:, :])

            # rotating staging + P3 + P9 buffers (zero borders persist)
            NBUF = 2
            NBUF9 = 3
            spbufs = []
            p9bufs = []
            for pi in range(NBUF):
                sp = sp_pool.tile([BPC * 3, SPLEN], f32r, tag=f"SP{pi}")
                spt0 = sp.tensor
                nc.vector.memset(
                    AP(tensor=spt0, offset=WROW - 1,
                       ap=[[SPLEN, BPC * 3], [WROW, RMAX], [1, 2]]).bitcast(f32),
                    0.0,
                )
                nc.vector.memset(sp[:, 0:WROW].bitcast(f32), 0.0)
                nc.vector.memset(sp[:, SPLEN - 1 : SPLEN].bitcast(f32), 0.0)
                spbufs.append(sp)
            for pi in range(NBUF9):
                p9 = p_pool.tile([BPC * 9 + 1, PWIN], f32r, tag=f"P9{pi}")
                nc.sync.dma_start(
                    out=p9[BPC * 9 : BPC * 9 + 1, :], in_=ones_p.ap()[0:1, :]
                )
                p9bufs.append(p9)

            def emit_in(s):
                h0, he = _slice_rows(s)
                ncols = (he - h0) * W
                xin = xin_pool.tile([NPART, 2 * RMAX * W], bf16, tag="xin")
                o = _SLICE_OFF[s]
                if s == 0:
                    # finer pieces so the first matmuls start sooner
                    for a0, a1 in ((0, 2048), (2048, ncols)):
                        nc.scalar.dma_start(
                            out=xin[:, a0:a1], in_=xp.ap()[:, o + a0 : o + a1]
                        )
                    for a0, a1 in ((0, 2048), (2048, ncols)):
                        nc.scalar.dma_start(
                            out=xin[:, ncols + a0 : ncols + a1],
                            in_=xp.ap()[:, o + ncols + a0 : o + ncols + a1],
                        )
                else:
                    nc.scalar.dma_start(
                        out=xin[:, :ncols], in_=xp.ap()[:, o : o + ncols]
                    )
                    nc.scalar.dma_start(
                        out=xin[:, ncols : 2 * ncols],
                        in_=xp.ap()[:, o + ncols : o + 2 * ncols],
                    )
                return xin

            def emit_cs_and_p(s, xin):
                hbase = SH * s - 1  # staging v-row 0 = image row hbase
                h0, he = _slice_rows(s)
                ncols = (he - h0) * W
                sp = spbufs[s % NBUF]
                spt = sp.tensor
                p9 = p9bufs[s % NBUF9]

                if s == NS - 1:
                    # bottom border: zero staging rows beyond image row 127
                    vz = (H - hbase) * WROW
                    nc.vector.memset(sp[:, vz:SPLEN].bitcast(f32), 0.0)

                # channel sum: ones^T @ [xhi; xlo], PSUM -> padded staging
                nchunks = (ncols + 511) // 512
                for ci in range(nchunks):
                    c0 = ci * 512
                    cn = min(512, ncols - c0)
                    nrows = cn // W
                    ps = cs_psum.tile([BPC * 3, 4, W], f32, tag="cs")
                    nc.tensor.matmul(
                        ps[:, :nrows, :],
                        ones_t[:, :],
                        xin[:, c0 : c0 + cn],
                        start=True,
                        stop=False,
                    )
                    nc.tensor.matmul(
                        ps[:, :nrows, :],
                        ones_t[:, :],
                        xin[:, ncols + c0 : ncols + c0 + cn],
                        start=False,
                        stop=True,
                    )
                    v0 = (h0 + 4 * ci - hbase) * WROW + 1
                    dst = AP(
                        tensor=spt,
                        offset=v0,
                        ap=[[SPLEN, BPC * 3], [WROW, nrows], [1, W]],
                    )
                    src = ps[:, :nrows, :]
                    if ci % 2 == 0:
                        nc.vector.tensor_copy(dst, src)
                    else:
                        nc.scalar.copy(dst, src)

                # build P9 single-hop: one DMA per (i,jj), both batches at
                # once (dst partitions 3i+jj and 9+3i+jj, stride 9).
                # P9[b*9+3i+jj, u] = sp[b*3+i, i... shifted]:
                #   = xp_b[32s*130 + u + jj*130 + (2-i)]
                spt_ = sp.tensor
                p9t = p9.tensor
                dmae = [nc.gpsimd, nc.gpsimd, nc.scalar]
                for i in range(3):
                    for jj in range(3):
                        m = 3 * i + jj
                        dmae[m % 3].dma_start(
                            out=AP(
                                tensor=p9t,
                                offset=m * PWIN,
                                ap=[[9 * PWIN, BPC], [1, PWIN]],
                            ),
                            in_=AP(
                                tensor=spt_,
                                offset=i * SPLEN + jj * WROW + 2 - i,
                                ap=[[3 * SPLEN, BPC], [1, PWIN]],
                            ),
                            single_packet=True,
                        )
                return p9

            def emit_warm():
                # dep-free matmuls that the PE chews on while waiting for a
                # P9 chain; keeps the HAM clock-gate at full rate.
                for _ in range(6):
                    ps = cs_psum.tile([BPC * 3, 4, W], f32, tag="cs")
                    nc.tensor.matmul(
                        ps[:, :, :],
                        ones_t[:, :],
                        xins[0][:, 0:512],
                        start=True,
                        stop=True,
                    )

            def emit_cv_and_out(s, p9):
                # conv: one K=20 fp32r matmul per 3-row chunk + psum->yt->hbm
                yt = y_pool.tile([NOUT, SH, W], f32, tag="yout")
                nchunk = (SH + 2) // 3
                for c in range(nchunk):
                    rr0 = c * 3
                    nrr = min(3, SH - rr0)
                    nn = nrr * WROW
                    ps = cv_psum.tile([NOUT, 3, WROW], f32, tag="cv")
                    nc.tensor.matmul(
                        ps[:, :nrr, :],
                        wb_t[:, :],
                        p9[:, rr0 * WROW : rr0 * WROW + nn],
                        start=True,
                        stop=True,
                    )
                    if c % 2 == 0:
                        nc.vector.tensor_copy(
                            yt[:, rr0 : rr0 + nrr, :], ps[:, :nrr, 0:W]
                        )
                    else:
                        nc.scalar.copy(yt[:, rr0 : rr0 + nrr, :], ps[:, :nrr, 0:W])

                half = SH // 2
                nc.sync.dma_start(
                    out=y.ap()[:, SH * s * W : (SH * s + half) * W],
                    in_=yt[:, :half, :],
                )
                nc.sync.dma_start(
                    out=y.ap()[:, (SH * s + half) * W : SH * (s + 1) * W],
                    in_=yt[:, half:, :],
                )

            # software-pipelined emission, two cs-stages ahead: PE stream is
            # cs0 cs1 cs2 cv0 cs3 cv1 cv2 cv3 so conv never heads the queue
            # while its P-build chain is still in flight.  Input DMAs are
            # emitted one slice ahead so they never queue behind P-chain
            # waits on their engine.
            DEPTH = 2
            p9s = {}
            xins = {s: emit_in(s) for s in range(NS)}
            for s in range(NS + DEPTH):
                if s < NS:
                    p9s[s] = emit_cs_and_p(s, xins[s])
                if s >= DEPTH:
                    emit_warm()
                    emit_cv_and_out(s - DEPTH, p9s[s - DEPTH])

    nc.compile()
    return nc


def _host_prep(x, weight, bias):
    bf = ml_dtypes.bfloat16
    wsum = weight.sum(axis=1)  # [COUT, 3, 3]
    wb = np.zeros((BPC * 9 + 1, NOUT), np.float32)
    for b in range(BPC):
        for i in range(3):
            for jj in range(3):
                wb[b * 9 + i * 3 + jj, b * COUT : (b + 1) * COUT] = wsum[
                    :, 2 - jj, i
                ]
    wb[BPC * 9, :] = np.tile(bias, BPC)
    ones_cs = np.zeros((NPART, BPC * 3), np.float32)
    for b in range(BPC):
        ones_cs[b * CIN : (b + 1) * CIN, b * 3 : (b + 1) * 3] = 1.0
    ones_cs = ones_cs.astype(bf)
    ones_p = np.ones((1, PWIN), np.float32)

    in_maps = []
    for r in range(N_CORES):
        xs = np.ascontiguousarray(
            x[r * BPC : (r + 1) * BPC].reshape(NPART, H, W)
        ).astype(np.float32)
        xhi = xs.astype(bf)
        xlo = (xs - xhi.astype(np.float32)).astype(bf)
        xpack = np.empty((NPART, XPACK_LEN), dtype=bf)
        for s in range(NS):
            h0, he = _slice_rows(s)
            n = (he - h0) * W
            o = _SLICE_OFF[s]
            xpack[:, o : o + n] = xhi[:, h0:he].reshape(NPART, n)
            xpack[:, o + n : o + 2 * n] = xlo[:, h0:he].reshape(NPART, n)
        in_maps.append(
            {
                "xpack": xpack,
                "ones_cs": ones_cs,
                "wb": wb,
                "ones_p": ones_p,
            }
        )
    return in_maps


def kernel(x, weight, bias):
    from concourse.bass_utils import run_bass_kernel_spmd

    x = np.asarray(x)
    weight = np.asarray(weight)
    bias = np.asarray(bias)
    nc = _build()
    in_maps = _host_prep(x, weight, bias)
    res = run_bass_kernel_spmd(nc, in_maps, core_ids=list(range(N_CORES)))
    out = np.concatenate(
        [
            res.results[r]["y"].reshape(BPC, COUT, H, W)
            for r in range(N_CORES)
        ],
        axis=0,
    )
    return out.astype(np.float32)



# revision 5
# speedup vs baseline: 1.1424x; 1.1424x over previous
"""FFTConv2d kernel for trn2, 8 NeuronCores.

Math: reference einsum 'bchw,oihw->bohw' factorizes:
  Y[b,o] = conv_full(sum_c x[b,c], sum_i w[o,i])[1:-1,1:-1] + bias[o]
i.e. a single-channel 3x3 "same" convolution (flipped kernel) per (b,o).

v2: fp16 end-to-end (input, staging, conv rhs/weights, output; PSUM fp32).
Per core (2 batches):
  1. DMA x slice in as fp16, partitions=(b,c), 34 row-slots per slice with
     zero-rows at image edges (memset), so all slices are uniform.
  2. Channel-sum via PE matmul with ones lhsT, 4x col-tiled (tile_position
     (0,32g)): phase g covers 8 output rows (10 input slots), psum
     partitions 32g+(b,i) so the psum->staging copy is [128, 512]-shaped.
  3. Copy PSUM -> padded staging fp16 [used parts 32g+3b+i, 10*130+2]
     (row stride 130, zero pad cols memset once per buffer).
  4. Build P9 [19, 32*130] fp16: one DMA per tap (u,v), 3-dim AP covering
     both batches and all 4 phase groups. Partition 18 holds ones (bias).
  5. Conv: per 3-row chunk one fp16 matmul wb[19,128].T @ P9 window ->
     PSUM [128,3,130]; bias rides the ones row.
  6. Copy PSUM -> yt fp16 (drop 2 pad cols), DMA yt -> HBM; host upcasts.
Pipelined over NS=4 row-slices, conv 2 slices behind channel-sum.
"""

import os
import sys
from functools import lru_cache

import numpy as np

for _p in ("/opt/trn_rl_repo", "/root/.axon_site/_ro/trn_rl_repo"):
    if os.path.isdir(_p) and _p not in sys.path:
        sys.path.insert(0, _p)

import ml_dtypes

B, CIN, COUT, H, W = 16, 64, 64, 128, 128
N_CORES = 8
BPC = B // N_CORES  # batches per core = 2
NS = 4  # row slices per core
SH = H // NS  # output rows per slice = 32
WROW = W + 2  # padded row stride = 130
PWIN = SH * WROW  # conv output window per slice = 4160
NPH = 4  # col-tile phases per slice
PROWS = SH // NPH  # output rows per phase = 8
PSLOT = PROWS + 2  # input slots per phase = 10
PHLEN = PSLOT * WROW + 2  # staging cols = 1302
NSLOT = SH + 2  # input slots per slice = 34
XCOLS = NSLOT * W  # xin cols per slice = 4352
NPART = BPC * CIN  # 128 input partitions (b, c)
NOUT = BPC * COUT  # 128 output partitions (b, o)
KCONV = BPC * 9 + 1  # 19 conv contraction rows

# xpack: per-slice contiguous fp16 rows [32s-1, 32s+33) clipped to [0, H)
_SLICE_ROWS = []
_SLICE_OFF = []
_off = 0
for _s in range(NS):
    _h0 = max(0, SH * _s - 1)
    _he = min(H, SH * _s + SH + 1)
    _SLICE_ROWS.append((_h0, _he))
    _SLICE_OFF.append(_off)
    _off += (_he - _h0) * W
XPACK_LEN = _off  # 17152


@lru_cache(maxsize=1)
def _build():
    import concourse.bacc as bacc
    import concourse.mybir as mybir
    import concourse.tile as tile
    from concourse.ap import AP

    f32 = mybir.dt.float32
    f16 = mybir.dt.float16

    nc = bacc.Bacc("TRN2", target_bir_lowering=False, debug=False, num_devices=N_CORES)

    xp = nc.dram_tensor("xpack", [NPART, XPACK_LEN], f16, kind="ExternalInput")
    ones_cs = nc.dram_tensor("ones_cs", [NPART, BPC * 3], f16, kind="ExternalInput")
    wb = nc.dram_tensor("wb", [KCONV, NOUT], f16, kind="ExternalInput")
    ones_p = nc.dram_tensor("ones_p", [1, PWIN], f16, kind="ExternalInput")
    y = nc.dram_tensor("y", [NOUT, H * W], f16, kind="ExternalOutput")

    with tile.TileContext(nc) as tc:
        with (
            tc.tile_pool(name="xin", bufs=4) as xin_pool,
            tc.tile_pool(name="sp", bufs=1) as sp_pool,
            tc.tile_pool(name="pbuf", bufs=1) as p_pool,
            tc.tile_pool(name="yout", bufs=2) as y_pool,
            tc.tile_pool(name="consts", bufs=1) as c_pool,
            tc.tile_pool(name="cs_ps", bufs=2, space="PSUM") as cs_psum,
            tc.tile_pool(name="cv_ps", bufs=2, space="PSUM") as cv_psum,
        ):
            ones_t = c_pool.tile([NPART, BPC * 3], f16, tag="ones_cs")
            nc.scalar.dma_start(out=ones_t[:, :], in_=ones_cs.ap()[:, :])
            wb_t = c_pool.tile([KCONV, NOUT], f16, tag="wb")
            nc.scalar.dma_start(out=wb_t[:, :], in_=wb.ap()[:, :])

            # persistent staging + P9 buffers (zero pads persist)
            NBUF = 2
            NBUF9 = 3
            spbufs = []
            p9bufs = []
            for pi in range(NBUF):
                sp = sp_pool.tile([NPART, PHLEN], f16, tag=f"SP{pi}")
                spt = sp.tensor
                # left pad col 0
                nc.vector.memset(sp[:, 0:1], 0.0)
                # right/left pad pairs {v*130+129, v*130+130} and tail col
                nc.vector.memset(
                    AP(tensor=spt, offset=WROW - 1,
                       ap=[[PHLEN, NPART], [WROW, PSLOT], [1, 2]]),
                    0.0,
                )
                nc.vector.memset(sp[:, PHLEN - 1 : PHLEN], 0.0)
                spbufs.append(sp)
            for pi in range(NBUF9):
                p9 = p_pool.tile([KCONV, PWIN], f16, tag=f"P9{pi}")
                nc.sync.dma_start(
                    out=p9[KCONV - 1 : KCONV, :], in_=ones_p.ap()[0:1, :]
                )
                p9bufs.append(p9)

            def emit_in(s):
                h0, he = _SLICE_ROWS[s]
                ncols = (he - h0) * W
                xin = xin_pool.tile([NPART, XCOLS], f16, tag="xin")
                o = _SLICE_OFF[s]
                # dst col of image row r is (r - (32s-1))*W; zero-row at edges
                d0 = (h0 - (SH * s - 1)) * W  # 128 for s=0 else 0
                if s == 0:
                    nc.vector.memset(xin[:, 0:W], 0.0)
                if s == NS - 1:
                    nc.vector.memset(xin[:, XCOLS - W : XCOLS], 0.0)
                nc.scalar.dma_start(
                    out=xin[:, d0 : d0 + ncols], in_=xp.ap()[:, o : o + ncols]
                )
                return xin

            CS_CHUNKS = [(0, 4), (4, 4), (8, 2)]  # (slot0, nrows) per bank

            def emit_cs_and_p(s, xin):
                sp = spbufs[s % NBUF]
                spt = sp.tensor
                p9 = p9bufs[s % NBUF9]

                pss = []
                for cb, (sl0, nrows) in enumerate(CS_CHUNKS):
                    n = nrows * W
                    ps = cs_psum.tile([NPART, nrows, W], f32, tag=f"cs{cb}")
                    pss.append(ps)
                    for g in range(NPH):
                        c0 = (g * PROWS + sl0) * W
                        nc.tensor.matmul(
                            ps[32 * g : 32 * g + BPC * 3, :, :],
                            ones_t[:, :],
                            xin[:, c0 : c0 + n],
                            start=True,
                            stop=True,
                            tile_position=(0, 32 * g),
                        )
                # psum -> staging (cast fp32 -> fp16), one copy per bank
                for cb, (sl0, nrows) in enumerate(CS_CHUNKS):
                    dst = AP(
                        tensor=spt,
                        offset=sl0 * WROW + 1,
                        ap=[[PHLEN, NPART], [WROW, nrows], [1, W]],
                    )
                    src = pss[cb][:, :, :]
                    if cb == 1:
                        nc.vector.tensor_copy(dst, src)
                    else:
                        nc.scalar.copy(dst, src)

                # P9 build: one DMA per (tap, batch); src partition-crossing
                # only in the outermost (phase) dim, dst single-partition.
                for u in range(3):
                    for v in range(3):
                        for b in range(BPC):
                            m = 9 * b + 3 * u + v
                            nc.gpsimd.dma_start(
                                out=p9[m : m + 1, 0:PWIN],
                                in_=AP(
                                    tensor=spt,
                                    offset=(3 * b + u) * PHLEN
                                    + (2 - u) * WROW + (2 - v),
                                    ap=[[32 * PHLEN, NPH], [1, PROWS * WROW]],
                                ),
                                single_packet=True,
                            )
                return p9

            def emit_cv_and_out(s, p9):
                yt = y_pool.tile([NOUT, SH, W], f16, tag="yout")
                nchunk = (SH + 2) // 3  # 11
                for c in range(nchunk):
                    rr0 = c * 3
                    nrr = min(3, SH - rr0)
                    ps = cv_psum.tile([NOUT, 3, WROW], f32, tag="cv")
                    nc.tensor.matmul(
                        ps[:, :nrr, :],
                        wb_t[:, :],
                        p9[:, rr0 * WROW : (rr0 + nrr) * WROW],
                        start=True,
                        stop=True,
                    )
                    if c % 2 == 0:
                        nc.vector.tensor_copy(
                            yt[:, rr0 : rr0 + nrr, :], ps[:, :nrr, 0:W]
                        )
                    else:
                        nc.scalar.copy(yt[:, rr0 : rr0 + nrr, :], ps[:, :nrr, 0:W])

                half = SH // 2
                nc.sync.dma_start(
                    out=y.ap()[:, SH * s * W : (SH * s + half) * W],
                    in_=yt[:, :half, :],
                )
                nc.sync.dma_start(
                    out=y.ap()[:, (SH * s + half) * W : SH * (s + 1) * W],
                    in_=yt[:, half:, :],
                )

            DEPTH = 2
            p9s = {}
            xins = {s: emit_in(s) for s in range(NS)}
            for s in range(NS + DEPTH):
                if s < NS:
                    p9s[s] = emit_cs_and_p(s, xins[s])
                if s >= DEPTH:
                    emit_cv_and_out(s - DEPTH, p9s[s - DEPTH])

    nc.compile()
    return nc


def _host_prep(x, weight, bias):
    fh = np.float16
    wsum = weight.sum(axis=1)  # [COUT, 3, 3] fp32
    wb = np.zeros((KCONV, NOUT), np.float32)
    for b in range(BPC):
        for u in range(3):
            for v in range(3):
                wb[b * 9 + 3 * u + v, b * COUT : (b + 1) * COUT] = wsum[:, u, v]
    wb[KCONV - 1, :] = np.tile(bias, BPC)
    wb = wb.astype(fh)
    ones_cs = np.zeros((NPART, BPC * 3), np.float32)
    for b in range(BPC):
        ones_cs[b * CIN : (b + 1) * CIN, b * 3 : (b + 1) * 3] = 1.0
    ones_cs = ones_cs.astype(fh)
    ones_p = np.ones((1, PWIN), dtype=fh)

    in_maps = []
    for r in range(N_CORES):
        xs = np.ascontiguousarray(
            x[r * BPC : (r + 1) * BPC].reshape(NPART, H, W)
        ).astype(fh)
        xpack = np.empty((NPART, XPACK_LEN), dtype=fh)
        for s in range(NS):
            h0, he = _SLICE_ROWS[s]
            n = (he - h0) * W
            o = _SLICE_OFF[s]
            xpack[:, o : o + n] = xs[:, h0:he].reshape(NPART, n)
        in_maps.append(
            {
                "xpack": xpack,
                "ones_cs": ones_cs,
                "wb": wb,
                "ones_p": ones_p,
            }
        )
    return in_maps


def kernel(x, weight, bias):
    from concourse.bass_utils import run_bass_kernel_spmd

    x = np.asarray(x)
    weight = np.asarray(weight)
    bias = np.asarray(bias)
    nc = _build()
    in_maps = _host_prep(x, weight, bias)
    res = run_bass_kernel_spmd(nc, in_maps, core_ids=list(range(N_CORES)))
    out = np.concatenate(
        [
            res.results[r]["y"].astype(np.float32).reshape(BPC, COUT, H, W)
            for r in range(N_CORES)
        ],
        axis=0,
    )
    return out
